# revision 13
# baseline (speedup 1.0000x reference)
"""AudioEncoder Trainium2 kernel: 8 NeuronCores.

Sharding: 4 batches x 2-core pairs. Within a pair, each core owns half the
sequence (512 of 1024 tokens). Per layer, the pair exchanges bf16 activations
with a 2-rank AllGather so both cores can compute full-sequence K/V; Q,
attention (its 512 queries x all 1024 keys), output proj, FFN and layernorms
are token-local. The conv stem is computed locally (host passes each core a
padded input window so its half appears at fixed columns -> identical SPMD
graph on all cores).

Dataflow is "transposed": activations live as [E(partitions), tokens(free)]
so every matmul contracts the partition dim with no activation transposes.
Layernorm stats use ones-vector matmuls (partition reduction on the PE) and
DMA partition-broadcast for the per-token scale/shift. Softmax skips the max
subtraction (logits are bounded ~ +-25 here; exp stays finite in f32).
Matmuls run in bf16 (weights pre-cast on host), accumulation f32; the f32
residual stream stays on-chip.
"""

import sys

import numpy as np
import ml_dtypes

for _p in ("/opt/trn_rl_repo",):
    if _p not in sys.path:
        sys.path.insert(0, _p)

import concourse.bass as bass  # noqa: E402
import concourse.mybir as mybir  # noqa: E402
import concourse.tile as tile  # noqa: E402
from concourse import bacc  # noqa: E402
from concourse.bass_utils import run_bass_kernel_spmd  # noqa: E402
from concourse.masks import make_identity  # noqa: E402

F32 = mybir.dt.float32
BF16 = mybir.dt.bfloat16
AF = mybir.ActivationFunctionType
ALU = mybir.AluOpType
BF = ml_dtypes.bfloat16

B, NMEL, TIN = 4, 80, 2048
T, E, H, L, M, C = 1024, 768, 12, 6, 3072, 527
HD = 64
EC, MC = 6, 24          # chunks of 128 along E / M
TL = 512                # tokens owned per core
KC = 8                  # k-token chunks of 128 over T
NCORES = 8
RG = [[0, 1], [2, 3], [4, 5], [6, 7]]
XPW = 1028              # padded conv input window width


def _r6(ap):
    return ap.rearrange("(c p) n -> p c n", p=128)


def build():
    nc = bacc.Bacc("TRN2", target_bir_lowering=False, debug=False,
                   num_devices=NCORES)
    dp = nc.declare_dram_parameter
    xp_d = dp("xp", [NMEL, XPW], F32, False)
    posT_d = dp("posT", [E, TL], F32, False)
    w1c_d = dp("w1c", [3, NMEL, E], BF16, False)
    b1c_d = dp("b1c", [128, EC], F32, False)
    w2c_d = dp("w2c", [3, E, E], BF16, False)
    b2c_d = dp("b2c", [128, EC], F32, False)
    wq_d = dp("wq", [L, E, E], BF16, False)
    wk_d = dp("wk", [L, E, E], BF16, False)
    wv_d = dp("wv", [L, E, E], BF16, False)
    wo_d = dp("wo", [L, E, E], BF16, False)
    w1_d = dp("w1", [L, E, M], BF16, False)
    w2_d = dp("w2", [L, M, E], BF16, False)
    bq_d = dp("bq", [L, 128, EC], F32, False)
    bk_d = dp("bk", [L, 128, EC], F32, False)
    bv_d = dp("bv", [L, 128, EC], F32, False)
    bo_d = dp("bo", [L, 128, EC], F32, False)
    b1_d = dp("b1", [L, 128, MC], F32, False)
    b2_d = dp("b2", [L, 128, EC], F32, False)
    g1_d = dp("g1", [L, 128, EC], F32, False)
    be1_d = dp("be1", [L, 128, EC], F32, False)
    g2_d = dp("g2", [L, 128, EC], F32, False)
    be2_d = dp("be2", [L, 128, EC], F32, False)
    out_d = dp("out", [E, TL], F32, True)

    cc_in = [nc.dram_tensor(f"cc_in{i}", [E, TL], BF16) for i in range(L)]
    cc_out = [
        nc.dram_tensor(f"cc_out{i}", [2 * E, TL], BF16) for i in range(L)
    ]

    from contextlib import ExitStack

    with tile.TileContext(nc) as tc, ExitStack() as es:
        def pool(**kw):
            return es.enter_context(tc.tile_pool(**kw))

        # ---- PSUM pools (8 banks total) ----
        psq = pool(name="psq", bufs=3, space="PSUM")
        pss = pool(name="pss", bufs=2, space="PSUM")
        pso = pool(name="pso", bufs=1, space="PSUM")
        psst = pool(name="psst", bufs=1, space="PSUM")
        pstr = pool(name="pstr", bufs=1, space="PSUM")
        # ---- SBUF pools needed by the stem (persistent) ----
        const = pool(name="const", bufs=1)
        xsp = pool(name="xsp", bufs=2)   # f32 stream
        xbp = pool(name="xbp", bufs=2)   # bf16 stream
        scp = pool(name="scp", bufs=2)   # f32 scratch rows

        drs = pool(name="drs", bufs=4, space="DRAM")

        def bcast(dst, src):
            """Broadcast src [1, N] (SBUF) to dst [P, N] via a DRAM bounce."""
            p, n = dst.shape
            d = drs.tile([1, n], F32, tag="dr", name=f"dr{id(dst) % 9999}")
            nc.gpsimd.dma_start(d[:], src)
            dap = d[:]
            nc.gpsimd.dma_start(
                dst, bass.AP(tensor=dap.tensor, offset=dap.offset,
                             ap=[[0, p]] + [list(x) for x in dap.ap][1:]))

        ones_bf = const.tile([128, 1], BF16, tag="ones")
        nc.vector.memset(ones_bf[:], 1.0)
        ident = const.tile([128, 128], BF16, tag="ident")
        make_identity(nc, ident[:])
        eps_t = const.tile([1, 1], F32, tag="eps")
        nc.vector.memset(eps_t[:], 1e-5)

        def ln(u, g_sb, b_sb, xs_out, xb_out):
            """xs_out = LN(u)*g+b (f32); xb_out = bf16 copy."""
            ub = atp.tile([128, EC, TL], BF16, tag="at")
            usq = qup.tile([128, EC, TL], BF16, tag="qu")
            for c in range(EC):
                nc.vector.tensor_copy(ub[:, c, :], u[:, c, :])
            for c in range(EC):
                nc.vector.tensor_mul(usq[:, c, :], ub[:, c, :], ub[:, c, :])
            sps = psst.tile([1, TL], F32, tag="pst")
            for c in range(EC):
                nc.tensor.matmul(sps[:], ones_bf[:], ub[:, c, :],
                                 start=(c == 0), stop=(c == EC - 1))
            m = smp.tile([1, TL], F32, tag="sm")
            nc.vector.tensor_scalar_mul(m[:], sps[:], 1.0 / E)
            sqs = psst.tile([1, TL], F32, tag="pst")
            for c in range(EC):
                nc.tensor.matmul(sqs[:], ones_bf[:], usq[:, c, :],
                                 start=(c == 0), stop=(c == EC - 1))
            msq = smp.tile([1, TL], F32, tag="sm")
            nc.vector.tensor_mul(msq[:], m[:], m[:])
            var = smp.tile([1, TL], F32, tag="sm")
            nc.vector.scalar_tensor_tensor(var[:], sqs[:], 1.0 / E, msq[:],
                                           op0=ALU.mult, op1=ALU.subtract)
            sd = smp.tile([1, TL], F32, tag="sm")
            nc.scalar.activation(sd[:], var[:], AF.Sqrt, bias=eps_t[:])
            rstd = smp.tile([1, TL], F32, tag="sm")
            nc.vector.reciprocal(rstd[:], sd[:])
            cneg = smp.tile([1, TL], F32, tag="sm")
            nc.vector.scalar_tensor_tensor(cneg[:], m[:], -1.0, rstd[:],
                                           op0=ALU.mult, op1=ALU.mult)
            ab = bcp.tile([128, TL], F32, tag="bc")
            cb = bcp.tile([128, TL], F32, tag="bc")
            bcast(ab[:], rstd[:])
            bcast(cb[:], cneg[:])
            for c in range(EC):
                t1 = scp.tile([128, TL], F32, tag="t1")
                nc.vector.tensor_mul(t1[:], u[:, c, :], ab[:])
                nc.vector.tensor_add(t1[:], t1[:], cb[:])
                nc.vector.tensor_scalar(xs_out[:, c, :], t1[:],
                                        g_sb[:, c:c + 1], b_sb[:, c:c + 1],
                                        op0=ALU.mult, op1=ALU.add)
                nc.vector.tensor_copy(xb_out[:, c, :], xs_out[:, c, :])

        # ================= conv stem =================
        with tc.tile_pool(name="stem", bufs=1) as stp:
            xpf = stp.tile([NMEL, XPW], F32, tag="st_xpf")
            nc.sync.dma_start(xpf[:], xp_d[:])
            xpb = stp.tile([NMEL, XPW], BF16, tag="st_xpb")
            nc.vector.tensor_copy(xpb[:], xpf[:])
            w1c_sb = stp.tile([NMEL, 3, E], BF16, tag="st_w1c")
            nc.sync.dma_start(w1c_sb[:], w1c_d[:].rearrange("k c e -> c k e"))
            b1c_sb = stp.tile([128, EC], F32, tag="st_b1c")
            nc.sync.dma_start(b1c_sb[:], b1c_d[:])
            b2c_sb = stp.tile([128, EC], F32, tag="st_b2c")
            nc.sync.dma_start(b2c_sb[:], b2c_d[:])
            w2c_sb = stp.tile([128, 3, EC, E], BF16, tag="st_w2c")
            nc.sync.dma_start(w2c_sb[:],
                              w2c_d[:].rearrange("k (c p) e -> p k c e", p=128))
            posT_sb = stp.tile([128, EC, TL], F32, tag="st_pos")
            nc.sync.dma_start(posT_sb[:], _r6(posT_d[:]))

            g1t = stp.tile([128, EC, 1026], BF16, tag="st_g1")
            for oc in range(EC):
                for (t0, tw) in ((0, 512), (512, 512), (1024, 2)):
                    ps = psq.tile([128, tw], F32, tag="pq")
                    for k in range(3):
                        nc.tensor.matmul(
                            ps[:], w1c_sb[:, k, oc * 128:(oc + 1) * 128],
                            xpb[:, t0 + k:t0 + k + tw],
                            start=(k == 0), stop=(k == 2))
                    nc.scalar.activation(g1t[:, oc, t0:t0 + tw], ps[:],
                                         AF.Gelu, bias=b1c_sb[:, oc:oc + 1])

            xs = xsp.tile([128, EC, TL], F32, tag="xs")
            xb = xbp.tile([128, EC, TL], BF16, tag="xb")
            for oc in range(EC):
                ps = psq.tile([128, TL], F32, tag="pq")
                n = 0
                for kc in range(EC):
                    for k in range(3):
                        nc.tensor.matmul(
                            ps[:], w2c_sb[:, k, kc, oc * 128:(oc + 1) * 128],
                            g1t[:, kc, k:k + 1024:2],
                            start=(n == 0), stop=(n == 17))
                        n += 1
                u0 = scp.tile([128, TL], F32, tag="t1")
                nc.scalar.activation(u0[:], ps[:], AF.Gelu,
                                     bias=b2c_sb[:, oc:oc + 1])
                nc.vector.tensor_add(xs[:, oc, :], u0[:], posT_sb[:, oc, :])
                nc.vector.tensor_copy(xb[:, oc, :], xs[:, oc, :])

        # ---- per-layer pools (allocated after the stem pool freed) ----
        bia = pool(name="bia", bufs=10)
        kup = pool(name="kup", bufs=2)   # kT / u1 / u2
        bap = pool(name="bap", bufs=1)   # vT / h_bf
        qup = pool(name="qup", bufs=2)   # qT / usq
        atp = pool(name="atp", bufs=2)   # attnT / ub
        xfp = pool(name="xfp", bufs=1)   # gathered x
        vap = pool(name="vap", bufs=2)   # v_aug per head
        ptp = pool(name="ptp", bufs=3)   # exp(S) per chunk
        bcp = pool(name="bcp", bufs=2)   # bcast rows
        smp = pool(name="smp", bufs=8)   # [1,TL] smalls
        wsp = pool(name="wsp", bufs=2)   # wq/wk/wv/wo
        wbp = pool(name="wbp", bufs=1)   # w1 / w2

        # ================= transformer layers =================
        for l in range(L):
            # ---- allgather the pair's halves (bf16) ----
            nc.gpsimd.dma_start(_r6(cc_in[l][:]), xb[:])
            nc.gpsimd.collective_compute(
                "AllGather", ALU.bypass, ins=[cc_in[l][:]],
                outs=[cc_out[l][:]], replica_groups=RG)
            xf = xfp.tile([128, EC, T], BF16, tag="xf")
            nc.sync.dma_start(xf[:, :, 0:TL], _r6(cc_out[l][0:E]))
            nc.sync.dma_start(xf[:, :, TL:T], _r6(cc_out[l][E:2 * E]))

            # ---- weights / biases ----
            wq_sb = wsp.tile([128, EC, E], BF16, tag="ws")
            nc.sync.dma_start(wq_sb[:], _r6(wq_d[l]))
            wk_sb = wsp.tile([128, EC, E], BF16, tag="ws")
            nc.sync.dma_start(wk_sb[:], _r6(wk_d[l]))
            wv_sb = wsp.tile([128, EC, E], BF16, tag="ws")
            nc.sync.dma_start(wv_sb[:], _r6(wv_d[l]))
            bq_sb = bia.tile([128, EC], F32, tag="bias")
            nc.sync.dma_start(bq_sb[:], bq_d[l])
            bk_sb = bia.tile([128, EC], F32, tag="bias")
            nc.sync.dma_start(bk_sb[:], bk_d[l])
            bv_sb = bia.tile([128, EC], F32, tag="bias")
            nc.sync.dma_start(bv_sb[:], bv_d[l])
            bo_sb = bia.tile([128, EC], F32, tag="bias")
            nc.sync.dma_start(bo_sb[:], bo_d[l])
            b1_sb = bia.tile([128, MC], F32, tag="bias")
            nc.sync.dma_start(b1_sb[:], b1_d[l])
            b2_sb = bia.tile([128, EC], F32, tag="bias")
            nc.sync.dma_start(b2_sb[:], b2_d[l])
            g1_sb = bia.tile([128, EC], F32, tag="bias")
            nc.sync.dma_start(g1_sb[:], g1_d[l])
            be1_sb = bia.tile([128, EC], F32, tag="bias")
            nc.sync.dma_start(be1_sb[:], be1_d[l])
            g2_sb = bia.tile([128, EC], F32, tag="bias")
            nc.sync.dma_start(g2_sb[:], g2_d[l])
            be2_sb = bia.tile([128, EC], F32, tag="bias")
            nc.sync.dma_start(be2_sb[:], be2_d[l])

            # ---- Q (local tokens) ----
            qT = qup.tile([128, EC, TL], BF16, tag="qu")
            for oc in range(EC):
                ps = psq.tile([128, TL], F32, tag="pq")
                for kc in range(EC):
                    nc.tensor.matmul(ps[:],
                                     wq_sb[:, kc, oc * 128:(oc + 1) * 128],
                                     xb[:, kc, :],
                                     start=(kc == 0), stop=(kc == EC - 1))
                nc.scalar.activation(qT[:, oc, :], ps[:], AF.Identity,
                                     bias=bq_sb[:, oc:oc + 1])

            # ---- K, V (full sequence) ----
            kT = kup.tile([128, EC, T], BF16, tag="ku")
            vT = bap.tile([128, EC, T], BF16, tag="ba")
            for (w_sb, b_sb, dstT) in ((wk_sb, bk_sb, kT), (wv_sb, bv_sb, vT)):
                for oc in range(EC):
                    for tc2 in range(2):
                        ps = psq.tile([128, TL], F32, tag="pq")
                        for kc in range(EC):
                            nc.tensor.matmul(
                                ps[:], w_sb[:, kc, oc * 128:(oc + 1) * 128],
                                xf[:, kc, tc2 * TL:(tc2 + 1) * TL],
                                start=(kc == 0), stop=(kc == EC - 1))
                        nc.scalar.activation(
                            dstT[:, oc, tc2 * TL:(tc2 + 1) * TL], ps[:],
                            AF.Identity, bias=b_sb[:, oc:oc + 1])

            wo_sb = wsp.tile([128, EC, E], BF16, tag="ws")
            nc.sync.dma_start(wo_sb[:], _r6(wo_d[l]))

            # ---- attention ----
            attnT = atp.tile([128, EC, TL], BF16, tag="at")
            for h in range(H):
                r0, hc = (h % 2) * 64, h // 2
                va = vap.tile([128, KC, 65], BF16, tag="va")
                nc.vector.memset(va[:, :, 64:65], 1.0)
                for kc in range(KC):
                    tp = pstr.tile([128, 64], BF16, tag="tr")
                    nc.tensor.transpose(
                        tp[:], vT[r0:r0 + 64, hc, kc * 128:(kc + 1) * 128],
                        ident[r0:r0 + 64, r0:r0 + 64])
                    nc.vector.tensor_copy(va[:, kc, 0:64], tp[:])
                po = pso.tile([65, TL], F32, tag="po")
                for kc in range(KC):
                    sp = pss.tile([128, TL], F32, tag="ps")
                    nc.tensor.matmul(
                        sp[:], kT[r0:r0 + 64, hc, kc * 128:(kc + 1) * 128],
                        qT[r0:r0 + 64, hc, :], start=True, stop=True)
                    pt = ptp.tile([128, TL], BF16, tag="pt")
                    nc.scalar.activation(pt[:], sp[:], AF.Exp)
                    nc.tensor.matmul(po[:], va[:, kc, :], pt[:],
                                     start=(kc == 0), stop=(kc == KC - 1))
                rden = smp.tile([1, TL], F32, tag="sm")
                nc.vector.reciprocal(rden[:], po[64:65, :])
                bc = bcp.tile([64, TL], F32, tag="bc")
                bcast(bc[:], rden[:])
                nc.vector.tensor_mul(attnT[r0:r0 + 64, hc, :],
                                     po[0:64, :], bc[:])

            # ---- output proj + residual ----
            u1 = kup.tile([128, EC, TL], F32, tag="ku")
            for oc in range(EC):
                ps = psq.tile([128, TL], F32, tag="pq")
                for kc in range(EC):
                    nc.tensor.matmul(ps[:],
                                     wo_sb[:, kc, oc * 128:(oc + 1) * 128],
                                     attnT[:, kc, :],
                                     start=(kc == 0), stop=(kc == EC - 1))
                nc.vector.scalar_tensor_tensor(
                    u1[:, oc, :], ps[:], bo_sb[:, oc:oc + 1], xs[:, oc, :],
                    op0=ALU.add, op1=ALU.add)

            ln1 = xsp.tile([128, EC, TL], F32, tag="xs")
            ln1b = xbp.tile([128, EC, TL], BF16, tag="xb")
            ln(u1, g1_sb, be1_sb, ln1, ln1b)

            # ---- FFN ----
            w1_sb = wbp.tile([128, EC, M], BF16, tag="wb")
            nc.sync.dma_start(w1_sb[:], _r6(w1_d[l]))
            hbf = bap.tile([128, MC, TL], BF16, tag="ba")
            for mc in range(MC):
                ps = psq.tile([128, TL], F32, tag="pq")
                for kc in range(EC):
                    nc.tensor.matmul(ps[:],
                                     w1_sb[:, kc, mc * 128:(mc + 1) * 128],
                                     ln1b[:, kc, :],
                                     start=(kc == 0), stop=(kc == EC - 1))
                nc.scalar.activation(hbf[:, mc, :], ps[:], AF.Relu,
                                     bias=b1_sb[:, mc:mc + 1])

            w2_sb = wbp.tile([128, MC, E], BF16, tag="wb")
            nc.sync.dma_start(w2_sb[:],
                              w2_d[l].rearrange("(c p) n -> p c n", p=128))
            u2 = kup.tile([128, EC, TL], F32, tag="ku")
            for oc in range(EC):
                ps = psq.tile([128, TL], F32, tag="pq")
                for kc in range(MC):
                    nc.tensor.matmul(ps[:],
                                     w2_sb[:, kc, oc * 128:(oc + 1) * 128],
                                     hbf[:, kc, :],
                                     start=(kc == 0), stop=(kc == MC - 1))
                nc.vector.scalar_tensor_tensor(
                    u2[:, oc, :], ps[:], b2_sb[:, oc:oc + 1], ln1[:, oc, :],
                    op0=ALU.add, op1=ALU.add)

            xs = xsp.tile([128, EC, TL], F32, tag="xs")
            xb = xbp.tile([128, EC, TL], BF16, tag="xb")
            ln(u2, g2_sb, be2_sb, xs, xb)

        nc.sync.dma_start(_r6(out_d[:]), xs[:])
    nc.compile()
    return nc


_NC_CACHE = None


def _get_nc():
    global _NC_CACHE
    if _NC_CACHE is None:
        _NC_CACHE = build()
    return _NC_CACHE


def _prep_inputs(inputs):
    """Host-side pre-pack: shard x, transpose/pre-cast weights."""
    f = {k: np.asarray(v, dtype=np.float32) for k, v in inputs.items()}

    def colchunks(v, n):  # (n*128,) -> (128, n)
        return np.ascontiguousarray(v.reshape(n, 128).T)

    com = {}
    com["w1c"] = np.ascontiguousarray(
        f["conv1_w"].transpose(2, 1, 0)).astype(BF)      # (3, 80, E)
    com["b1c"] = colchunks(f["conv1_b"], EC)
    com["w2c"] = np.ascontiguousarray(
        f["conv2_w"].transpose(2, 1, 0)).astype(BF)      # (3, E, E)
    com["b2c"] = colchunks(f["conv2_b"], EC)
    com["wq"] = np.ascontiguousarray(
        f["wq"].transpose(0, 2, 1, 3).reshape(L, E, E)).astype(BF)
    com["wk"] = np.ascontiguousarray(
        f["wk"].transpose(0, 2, 1, 3).reshape(L, E, E)).astype(BF)
    com["wv"] = np.ascontiguousarray(
        f["wv"].transpose(0, 2, 1, 3).reshape(L, E, E)).astype(BF)
    com["wo"] = np.ascontiguousarray(f["wo"]).astype(BF)
    com["w1"] = np.ascontiguousarray(f["w1"]).astype(BF)
    com["w2"] = np.ascontiguousarray(f["w2"]).astype(BF)
    for (nm, src, n) in (("bq", "bq", EC), ("bk", "bk", EC), ("bv", "bv", EC),
                         ("bo", "bo", EC), ("b1", "b1", MC), ("b2", "b2", EC),
                         ("g1", "ln1_g", EC), ("be1", "ln1_b", EC),
                         ("g2", "ln2_g", EC), ("be2", "ln2_b", EC)):
        arr = f[src].reshape(L, -1)
        com[nm] = np.ascontiguousarray(
            arr.reshape(L, n, 128).transpose(0, 2, 1))
    posT = np.ascontiguousarray(f["pos_emb"].T)          # (E, T)

    in_maps = []
    for core in range(NCORES):
        b, half = core // 2, core % 2
        xp = np.zeros((NMEL, XPW), np.float32)
        lo_abs = 1024 * half - 2
        src_lo, src_hi = max(0, lo_abs), min(TIN, lo_abs + XPW)
        dst_lo = src_lo - lo_abs
        xp[:, dst_lo:dst_lo + (src_hi - src_lo)] = f["x"][b][:, src_lo:src_hi]
        m = dict(com)
        m["xp"] = xp
        m["posT"] = np.ascontiguousarray(posT[:, TL * half:TL * half + TL])
        in_maps.append(m)
    return in_maps


def _head(inputs, xT_halves):
    """Final LN + classifier head on host (tiny: 32 MFLOP)."""
    f = {k: np.asarray(v, dtype=np.float32) for k, v in inputs.items()}
    cls = np.stack([xT_halves[2 * b][:, 0] for b in range(B)])  # (B, E)

    def _ln_np(x, g, bb):
        mu = x.mean(-1, keepdims=True)
        va = np.square(x - mu).mean(-1, keepdims=True)
        return (x - mu) / np.sqrt(va + 1e-5) * g + bb

    x = _ln_np(cls, f["lnp_g"], f["lnp_b"])
    x = _ln_np(x, f["hln_g"], f["hln_b"])
    logits = np.maximum(x @ f["hw1"] + f["hb1"], 0.0) @ f["hw2"] + f["hb2"]
    z = logits - logits.max(-1, keepdims=True)
    ez = np.exp(z)
    return (ez / ez.sum(-1, keepdims=True)).astype(np.float32)


def _run(inputs, trace=False):
    nc = _get_nc()
    in_maps = _prep_inputs(inputs)
    res = run_bass_kernel_spmd(nc, in_maps, list(range(NCORES)), trace=trace)
    outs = [res.results[i]["out"] for i in range(NCORES)]
    return _head(inputs, outs), res


def kernel(**inputs):
    out, _ = _run(inputs, trace=False)
    return out
